# revision 28
# baseline (speedup 1.0000x reference)
"""Trainium2 Bass kernel for dynamic kNN graph construction.

Problem: for each of N=16384 src embeddings (D=16), find its k=10 nearest
dst embeddings (L2), emit the edge list [2, N*k] plus batch-normalized,
sigmoid-squashed, mean-normalized edge weights [N*k, 1].

Strategy (8 NeuronCores, src rows sharded 2048/core, dst replicated):
  Device (per core, per [128 src x 16384 dst] row-block):
    - PE computes v = 2*s.d - |d|^2 (ordering-equivalent to -dist^2 per
      row) as a K=17 augmented float16 matmul -> PSUM f32.  fp16 is safe:
      the end-to-end value error is <0.06 and candidate selection was
      validated to tolerate 0.1 absolute noise; the host re-ranks
      candidates exactly afterwards.
    - ACT stages PSUM -> SBUF as fp16 (2048-wide copies); two groups per
      block are "half staged" with DVE reading one half from PSUM, which
      rebalances load from ACT (the bottleneck) onto DVE.
    - DVE reduces with a pairwise-max tree 16384 -> 8192 -> 4096 -> 2048
      -> 1024 (fp16 SBUF-only TensorTensor runs in the 2x_1p perf mode),
      then per-part top-8 (Max + MaxIndex) over 8 parts of 128 -> 64
      winner positions + values per row.
  Host:
    - Prune winners: a winner whose value is 0.25 below the row's k-th
      best winner value cannot contain a true top-k column (device value
      error << 0.25).  ~12 winners/row survive.
    - Expand each winner to its 16 possible origin columns (the max tree
      is value-only; trying all origins beats disambiguating on-device).
    - Exact refine: coarse f32 distance -> top-24, then rank by
      float64(s2_f32) + float64(d2_f32) - 2*dot_f64 (validated to
      reproduce the reference's f32 top-k order on its near-ties).
    - likelihood (f32 dots), BatchNorm1d batch stats, sigmoid, mean-norm.
"""

import numpy as np

N = 16384
D = 16
NCORES = 8
ROWS = N // NCORES        # src rows per core
NBLK = ROWS // 128        # row-blocks of 128 per core
KDIM = D + 1              # contraction: 16 features + d2 row
EPS = 1e-5
PRUNE_MARGIN = np.float32(0.25)

_nc_cache = None
_last_results = None      # test harness introspection


def _build_nc():
    global _nc_cache
    if _nc_cache is not None:
        return _nc_cache

    import concourse.bacc as bacc
    import concourse.mybir as mybir
    from concourse.tile import TileContext

    dt = mybir.dt
    # Bacc (not raw Bass): its compile() legalizes sync waits — hardware
    # instructions carry at most ONE wait; excess splits into EVSEM ops.
    nc = bacc.Bacc("TRN2", target_bir_lowering=False, debug=False)

    lhsT = nc.dram_tensor("lhst", [KDIM, ROWS], dt.float16, kind="ExternalInput")
    rhs = nc.dram_tensor("rhs", [KDIM, N], dt.float16, kind="ExternalInput")
    widx = nc.dram_tensor("widx", [ROWS, 64], dt.uint32, kind="ExternalOutput")
    wval = nc.dram_tensor("wval", [ROWS, 64], dt.float16, kind="ExternalOutput")

    with TileContext(nc) as tc:
        with (
            tc.tile_pool(name="consts", bufs=1) as cpool,
            tc.tile_pool(name="psum", bufs=1, space="PSUM") as ppool,
            tc.tile_pool(name="work", bufs=1) as wpool,
        ):
            rhs_sb = cpool.tile([KDIM, N], dt.float16, tag="rhs")
            lhs_sb = cpool.tile([KDIM, ROWS], dt.float16, tag="lhs")
            nc.sync.dma_start(lhs_sb[:], lhsT[:])
            # chunked so the first matmuls start before the whole dst
            # matrix has landed
            for c in range(8):
                nc.sync.dma_start(
                    rhs_sb[:, c * (N // 8):(c + 1) * (N // 8)],
                    rhs[:, c * (N // 8):(c + 1) * (N // 8)],
                )

            # Tiles are allocated once and rotated manually: pool-release
            # waits bypass Tile's wait minimization, manual rotation keeps
            # dependencies as plain data deps (one semaphore each).
            psum_tiles = [
                ppool.tile([128, 2048], dt.float32, tag=f"p{i}", name=f"p{i}")
                for i in range(2)
            ]
            m0_tiles = [
                wpool.tile([128, 16384], dt.float16, tag=f"m0_{i}", name=f"m0_{i}")
                for i in range(1)
            ]
            m1_tiles = [
                wpool.tile([128, 8192], dt.float16, tag=f"m1_{i}", name=f"m1_{i}")
                for i in range(2)
            ]
            m2_tiles = [
                wpool.tile([128, 4096], dt.float16, tag=f"m2_{i}", name=f"m2_{i}")
                for i in range(2)
            ]
            m3_tiles = [
                wpool.tile([128, 2048], dt.float16, tag=f"m3_{i}", name=f"m3_{i}")
                for i in range(2)
            ]
            m4_tiles = [
                wpool.tile([128, 1024], dt.float16, tag=f"m4_{i}", name=f"m4_{i}")
                for i in range(2)
            ]
            sg_tiles = [
                wpool.tile([128, 1024], dt.float32, tag=f"sg{i}", name=f"sg{i}")
                for i in range(2)
            ]
            val_tiles = [
                wpool.tile([128, 64], dt.float16, tag=f"v{i}", name=f"v{i}")
                for i in range(2)
            ]
            idx_tiles = [
                wpool.tile([128, 64], dt.uint32, tag=f"i{i}", name=f"i{i}")
                for i in range(2)
            ]

            # Two of the eight 2048-column groups per block are "half
            # staged": ACT copies only the upper 1024 (as f32) and DVE's L1
            # max reads the lower 1024 straight from PSUM — shifting load
            # from the bottleneck ACT engine onto DVE, which has headroom.
            # They sit late in the block so DVE has drained the previous
            # block's tree by the time their L1 gates the PSUM rotation.
            # Every L1 is emitted inline with its group so DVE's in-order
            # queue consumes groups at the same pace ACT stages them.
            HALF_STAGED = (6, 7)
            pcount = 0
            for b in range(NBLK):
                m0 = m0_tiles[0]
                m1 = m1_tiles[b % 2]
                m2 = m2_tiles[b % 2]
                m3 = m3_tiles[b % 2]
                m4 = m4_tiles[b % 2]
                vals = val_tiles[b % 2]
                idxs = idx_tiles[b % 2]
                w_b = lhs_sb[:, b * 128:(b + 1) * 128]
                for h in range(8):
                    pt = psum_tiles[pcount % 2]
                    for q in range(4):
                        col0 = h * 2048 + q * 512
                        nc.tensor.matmul(
                            pt[:, q * 512:(q + 1) * 512],
                            w_b,
                            rhs_sb[:, col0:col0 + 512],
                            start=True,
                            stop=True,
                        )
                    if h in HALF_STAGED:
                        sg = sg_tiles[HALF_STAGED.index(h)]
                        nc.scalar.copy(sg[:], pt[:, 1024:])
                        nc.vector.tensor_max(
                            m1[:, h * 1024:(h + 1) * 1024], pt[:, :1024], sg[:]
                        )
                    else:
                        # stage f32 PSUM -> fp16 SBUF on the scalar engine,
                        # then the fp16 L1 max (2x_1p) on DVE
                        nc.scalar.copy(m0[:, h * 2048:(h + 1) * 2048], pt[:])
                        nc.vector.tensor_max(
                            m1[:, h * 1024:(h + 1) * 1024],
                            m0[:, h * 2048:h * 2048 + 1024],
                            m0[:, h * 2048 + 1024:(h + 1) * 2048],
                        )
                    pcount += 1
                nc.vector.tensor_max(m2[:], m1[:, :4096], m1[:, 4096:])
                nc.vector.tensor_max(m3[:], m2[:, :2048], m2[:, 2048:])
                nc.vector.tensor_max(m4[:], m3[:, :1024], m3[:, 1024:])
                for p in range(8):
                    part = m4[:, p * 128:(p + 1) * 128]
                    nc.vector.max(vals[:, p * 8:(p + 1) * 8], part)
                    nc.vector.max_index(
                        idxs[:, p * 8:(p + 1) * 8], vals[:, p * 8:(p + 1) * 8], part
                    )
                nc.sync.dma_start(widx[b * 128:(b + 1) * 128, :], idxs[:])
                nc.sync.dma_start(wval[b * 128:(b + 1) * 128, :], vals[:])

    nc.compile()
    _nc_cache = nc
    return nc


def _expand_origins(u):
    """Winner position in M4 [*, W] -> 16 candidate dst columns [*, W*16].

    Tree mapping: M4[u] = max(M3[u], M3[u+1024]); M3[m] = max(M2[m],
    M2[m+2048]); M2[m] = max(M1[m], M1[m+4096]); M1[q] covers columns
    {2048g+o, 2048g+1024+o} with g = q//1024, o = q%1024.
    """
    origins = []
    for dm3 in (0, 1024):
        m3pos = u + dm3
        for dm2 in (0, 2048):
            m2pos = m3pos + dm2
            for dq in (0, 4096):
                q = m2pos + dq
                g = q // 1024
                o = q % 1024
                origins.append(2048 * g + o)
                origins.append(2048 * g + 1024 + o)
    return np.stack(origins, axis=2).reshape(u.shape[0], u.shape[1] * 16)


def kernel(src_embeddings, dst_embeddings, bn_gamma, bn_beta, k):
    global _last_results
    from concourse.bass_utils import run_bass_kernel_spmd

    src = np.ascontiguousarray(np.asarray(src_embeddings, dtype=np.float32))
    dst = np.ascontiguousarray(np.asarray(dst_embeddings, dtype=np.float32))
    gamma = np.float32(np.asarray(bn_gamma).reshape(-1)[0])
    beta = np.float32(np.asarray(bn_beta).reshape(-1)[0])
    k = int(np.asarray(k))
    assert src.shape == (N, D) and dst.shape == (N, D)
    assert 1 <= k <= 24, f"kernel supports k<=24, got {k}"

    # np.sum (pairwise) f32 rounding here matches the reference's jnp.sum —
    # validated on the near-tie pairs; einsum rounds differently and flips
    # the one razor-thin pair in this dataset.
    d2 = (dst * dst).sum(axis=-1).astype(np.float32)
    s2 = (src * src).sum(axis=-1).astype(np.float32)
    s2_64 = s2.astype(np.float64)
    d2_64 = d2.astype(np.float64)

    rhs = np.empty((KDIM, N), dtype=np.float16)
    rhs[:D] = dst.T.astype(np.float16)
    rhs[D] = (-d2).astype(np.float16)

    in_maps = []
    for c in range(NCORES):
        shard = src[c * ROWS:(c + 1) * ROWS]
        lhsT = np.empty((KDIM, ROWS), dtype=np.float16)
        lhsT[:D] = (2.0 * shard).T.astype(np.float16)
        lhsT[D] = np.ones(ROWS, dtype=np.float16)
        in_maps.append({"lhst": np.ascontiguousarray(lhsT), "rhs": rhs})

    nc = _build_nc()
    res = run_bass_kernel_spmd(nc, in_maps, core_ids=list(range(NCORES)))
    _last_results = res

    wpos = np.concatenate(
        [r["widx"].astype(np.int64) for r in res.results], axis=0
    )  # [N, 64] part-local positions in M4
    wval = np.concatenate(
        [r["wval"].astype(np.float32) for r in res.results], axis=0
    )  # [N, 64] winner values
    wpos = wpos + (np.arange(64, dtype=np.int64)[None, :] // 8) * 128

    # prune winners that cannot contain a true top-k column
    kth = min(k, 10)
    thr = -np.partition(-wval, kth - 1, axis=1)[:, kth - 1] - PRUNE_MARGIN
    keep = wval >= thr[:, None]

    M = max(24, k + 8)
    sel = np.empty((N, k), dtype=np.int64)
    lik = np.empty((N, k), dtype=np.float32)
    CH = 2048
    for r0 in range(0, N, CH):
        r1 = min(r0 + CH, N)
        kk = keep[r0:r1]
        nkeep = int(kk.sum(axis=1).max())
        order_w = np.argsort(~kk, axis=1, kind="stable")[:, :nkeep]
        wpos_k = np.take_along_axis(wpos[r0:r1], order_w, axis=1)
        cs = np.sort(_expand_origins(wpos_k), axis=1)
        dd = dst[cs]
        dot32 = np.matmul(dd, src[r0:r1, :, None])[:, :, 0].astype(np.float32)
        score32 = (s2[r0:r1, None] + d2[cs]) - 2.0 * dot32
        mM = min(M, score32.shape[1] - 1)
        topM = np.argpartition(score32, mM, axis=1)[:, :mM]
        topM.sort(axis=1)
        csM = np.take_along_axis(cs, topM, axis=1)
        ddM = dst[csM]
        dot64 = np.matmul(ddM.astype(np.float64), src[r0:r1, :, None].astype(np.float64))[:, :, 0]
        distM = (s2_64[r0:r1, None] + d2_64[csM]) - 2.0 * dot64
        order = np.argsort(distM, axis=1, kind="stable")[:, :k]
        sel[r0:r1] = np.take_along_axis(csM, order, axis=1)
        dot32M = np.matmul(ddM, src[r0:r1, :, None])[:, :, 0].astype(np.float32)
        lik[r0:r1] = np.take_along_axis(dot32M, order, axis=1)

    lik = lik.reshape(-1)
    mu = np.float32(lik.mean())
    var = np.float32(lik.var())
    logits = (lik - mu) * np.float32(1.0 / np.sqrt(var + np.float32(EPS))) * gamma + beta
    w = (1.0 / (1.0 + np.exp(-logits))).astype(np.float32)
    w = (w / np.float32(w.mean())).astype(np.float32)[:, None]

    graph = np.stack(
        [
            np.repeat(np.arange(N, dtype=np.int32), k),
            sel.reshape(-1).astype(np.int32),
        ],
        axis=0,
    )
    return graph, w


# revision 37
# speedup vs baseline: 1.0238x; 1.0238x over previous
"""Trainium2 Bass kernel for dynamic kNN graph construction.

Problem: for each of N=16384 src embeddings (D=16), find its k=10 nearest
dst embeddings (L2), emit the edge list [2, N*k] plus batch-normalized,
sigmoid-squashed, mean-normalized edge weights [N*k, 1].

Strategy (8 NeuronCores, src rows sharded 2048/core, dst replicated):
  Device (per core, per [128 src x 16384 dst] row-block):
    - PE computes v = 2*s.d - |d|^2 (ordering-equivalent to -dist^2 per
      row) as a K=17 augmented float16 matmul -> PSUM f32.  fp16 is safe:
      the end-to-end value error is <0.06 and candidate selection was
      validated to tolerate 0.1 absolute noise; the host re-ranks
      candidates exactly afterwards.
    - ACT stages PSUM -> SBUF as fp16 (2048-wide copies); two groups per
      block are "half staged" with DVE reading one half from PSUM, which
      rebalances load from ACT (the bottleneck) onto DVE.
    - DVE reduces with a pairwise-max tree 16384 -> 8192 -> 4096 -> 2048
      -> 1024 (fp16 SBUF-only TensorTensor runs in the 2x_1p perf mode),
      then per-part top-8 (Max + MaxIndex) over 8 parts of 128 -> 64
      winner positions + values per row.
  Host:
    - Prune winners: a winner whose value is 0.25 below the row's k-th
      best winner value cannot contain a true top-k column (device value
      error << 0.25).  ~12 winners/row survive.
    - Expand each winner to its 16 possible origin columns (the max tree
      is value-only; trying all origins beats disambiguating on-device).
    - Exact refine: coarse f32 distance -> top-24, then rank by
      float64(s2_f32) + float64(d2_f32) - 2*dot_f64 (validated to
      reproduce the reference's f32 top-k order on its near-ties).
    - likelihood (f32 dots), BatchNorm1d batch stats, sigmoid, mean-norm.
"""

import numpy as np

N = 16384
D = 16
NCORES = 8
ROWS = N // NCORES        # src rows per core
NBLK = ROWS // 128        # row-blocks of 128 per core
KDIM = D + 1              # contraction: 16 features + d2 row
EPS = 1e-5
PRUNE_MARGIN = np.float32(0.25)

_nc_cache = None
_last_results = None      # test harness introspection


def _build_nc():
    global _nc_cache
    if _nc_cache is not None:
        return _nc_cache

    import concourse.bacc as bacc
    import concourse.mybir as mybir
    from concourse.tile import TileContext

    dt = mybir.dt
    # Bacc (not raw Bass): its compile() legalizes sync waits — hardware
    # instructions carry at most ONE wait; excess splits into EVSEM ops.
    nc = bacc.Bacc("TRN2", target_bir_lowering=False, debug=False)

    lhsT = nc.dram_tensor("lhst", [KDIM, ROWS], dt.float16, kind="ExternalInput")
    rhs = nc.dram_tensor("rhs", [KDIM, N], dt.float16, kind="ExternalInput")
    m4out = nc.dram_tensor("m4out", [ROWS, 1024], dt.float16, kind="ExternalOutput")

    with TileContext(nc) as tc:
        with (
            tc.tile_pool(name="consts", bufs=1) as cpool,
            tc.tile_pool(name="psum", bufs=1, space="PSUM") as ppool,
            tc.tile_pool(name="work", bufs=1) as wpool,
        ):
            rhs_sb = cpool.tile([KDIM, N], dt.float16, tag="rhs")
            lhs_sb = cpool.tile([KDIM, ROWS], dt.float16, tag="lhs")
            nc.sync.dma_start(lhs_sb[:], lhsT[:])
            # chunked so the first matmuls start before the whole dst
            # matrix has landed
            for c in range(8):
                nc.sync.dma_start(
                    rhs_sb[:, c * (N // 8):(c + 1) * (N // 8)],
                    rhs[:, c * (N // 8):(c + 1) * (N // 8)],
                )

            # Tiles are allocated once and rotated manually: pool-release
            # waits bypass Tile's wait minimization, manual rotation keeps
            # dependencies as plain data deps (one semaphore each).
            psum_tiles = [
                ppool.tile([128, 2048], dt.float32, tag=f"p{i}", name=f"p{i}")
                for i in range(2)
            ]
            m0_tiles = [
                wpool.tile([128, 16384], dt.float16, tag=f"m0_{i}", name=f"m0_{i}")
                for i in range(1)
            ]
            m1_tiles = [
                wpool.tile([128, 8192], dt.float16, tag=f"m1_{i}", name=f"m1_{i}")
                for i in range(2)
            ]
            m2_tiles = [
                wpool.tile([128, 4096], dt.float16, tag=f"m2_{i}", name=f"m2_{i}")
                for i in range(2)
            ]
            m3_tiles = [
                wpool.tile([128, 2048], dt.float16, tag=f"m3_{i}", name=f"m3_{i}")
                for i in range(2)
            ]
            m4_tiles = [
                wpool.tile([128, 1024], dt.float16, tag=f"m4_{i}", name=f"m4_{i}")
                for i in range(2)
            ]
            sg_tiles = [
                wpool.tile([128, 1024], dt.float32, tag=f"sg{i}", name=f"sg{i}")
                for i in range(4)
            ]

            # Four of the eight 2048-column groups per block are "half
            # staged" to balance ACT (~189us) and DVE (~177us) busy time.
            # Half-staged groups sit late in the block so DVE has drained
            # the previous block's tree before their L1 gates the PSUM
            # rotation.  Every L1 is emitted inline with its group so DVE's
            # in-order queue consumes groups at the pace ACT stages them.
            HALF_STAGED = (4, 5, 6, 7)
            pcount = 0
            for b in range(NBLK):
                m0 = m0_tiles[0]
                m1 = m1_tiles[b % 2]
                m2 = m2_tiles[b % 2]
                m3 = m3_tiles[b % 2]
                m4 = m4_tiles[b % 2]
                w_b = lhs_sb[:, b * 128:(b + 1) * 128]
                for h in range(8):
                    pt = psum_tiles[pcount % 2]
                    for q in range(4):
                        col0 = h * 2048 + q * 512
                        nc.tensor.matmul(
                            pt[:, q * 512:(q + 1) * 512],
                            w_b,
                            rhs_sb[:, col0:col0 + 512],
                            start=True,
                            stop=True,
                        )
                    if h in HALF_STAGED:
                        # ACT stages only the upper half (f32); DVE's L1 max
                        # reads the lower half straight from PSUM, shifting
                        # staging load from ACT onto DVE
                        sg = sg_tiles[HALF_STAGED.index(h)]
                        nc.scalar.copy(sg[:], pt[:, 1024:])
                        nc.vector.tensor_max(
                            m1[:, h * 1024:(h + 1) * 1024], pt[:, :1024], sg[:]
                        )
                    else:
                        # stage f32 PSUM -> fp16 SBUF on the scalar engine,
                        # then the fp16 L1 max (2x_1p) on DVE
                        nc.scalar.copy(m0[:, h * 2048:(h + 1) * 2048], pt[:])
                        nc.vector.tensor_max(
                            m1[:, h * 1024:(h + 1) * 1024],
                            m0[:, h * 2048:h * 2048 + 1024],
                            m0[:, h * 2048 + 1024:(h + 1) * 2048],
                        )
                    pcount += 1
                nc.vector.tensor_max(m2[:], m1[:, :4096], m1[:, 4096:])
                nc.vector.tensor_max(m3[:], m2[:, :2048], m2[:, 2048:])
                nc.vector.tensor_max(m4[:], m3[:, :1024], m3[:, 1024:])
                nc.sync.dma_start(m4out[b * 128:(b + 1) * 128, :], m4[:])

    nc.compile()
    _nc_cache = nc
    return nc


def _expand_origins(u):
    """Winner position in M4 [*, W] -> 16 candidate dst columns [*, W*16].

    Tree mapping: M4[u] = max(M3[u], M3[u+1024]); M3[m] = max(M2[m],
    M2[m+2048]); M2[m] = max(M1[m], M1[m+4096]); M1[q] covers columns
    {2048g+o, 2048g+1024+o} with g = q//1024, o = q%1024.
    """
    origins = []
    for dm3 in (0, 1024):
        m3pos = u + dm3
        for dm2 in (0, 2048):
            m2pos = m3pos + dm2
            for dq in (0, 4096):
                q = m2pos + dq
                g = q // 1024
                o = q % 1024
                origins.append(2048 * g + o)
                origins.append(2048 * g + 1024 + o)
    return np.stack(origins, axis=2).reshape(u.shape[0], u.shape[1] * 16)


def kernel(src_embeddings, dst_embeddings, bn_gamma, bn_beta, k):
    global _last_results
    from concourse.bass_utils import run_bass_kernel_spmd

    src = np.ascontiguousarray(np.asarray(src_embeddings, dtype=np.float32))
    dst = np.ascontiguousarray(np.asarray(dst_embeddings, dtype=np.float32))
    gamma = np.float32(np.asarray(bn_gamma).reshape(-1)[0])
    beta = np.float32(np.asarray(bn_beta).reshape(-1)[0])
    k = int(np.asarray(k))
    assert src.shape == (N, D) and dst.shape == (N, D)
    assert 1 <= k <= 24, f"kernel supports k<=24, got {k}"

    # np.sum (pairwise) f32 rounding here matches the reference's jnp.sum —
    # validated on the near-tie pairs; einsum rounds differently and flips
    # the one razor-thin pair in this dataset.
    d2 = (dst * dst).sum(axis=-1).astype(np.float32)
    s2 = (src * src).sum(axis=-1).astype(np.float32)
    s2_64 = s2.astype(np.float64)
    d2_64 = d2.astype(np.float64)

    rhs = np.empty((KDIM, N), dtype=np.float16)
    rhs[:D] = dst.T.astype(np.float16)
    rhs[D] = (-d2).astype(np.float16)

    in_maps = []
    for c in range(NCORES):
        shard = src[c * ROWS:(c + 1) * ROWS]
        lhsT = np.empty((KDIM, ROWS), dtype=np.float16)
        lhsT[:D] = (2.0 * shard).T.astype(np.float16)
        lhsT[D] = np.ones(ROWS, dtype=np.float16)
        in_maps.append({"lhst": np.ascontiguousarray(lhsT), "rhs": rhs})

    nc = _build_nc()
    res = run_bass_kernel_spmd(nc, in_maps, core_ids=list(range(NCORES)))
    _last_results = res

    m4 = np.concatenate(
        [r["m4out"] for r in res.results], axis=0
    ).astype(np.float32)  # [N, 1024] reduced max values

    # keep every reduced position whose value is within PRUNE_MARGIN of the
    # row's k-th best (device value error << margin, so no true top-k origin
    # can hide below it); T caps the padded width, validated far above the
    # observed keep counts
    T = 32
    kth = min(k, 10)
    wpos = np.argpartition(-m4, T - 1, axis=1)[:, :T]
    wval = np.take_along_axis(m4, wpos, axis=1)
    thr = -np.partition(-wval, kth - 1, axis=1)[:, kth - 1] - PRUNE_MARGIN
    keep = wval >= thr[:, None]
    assert (keep.sum(axis=1) < T).all(), "prune width T too small"

    M = max(24, k + 8)
    sel = np.empty((N, k), dtype=np.int64)
    lik = np.empty((N, k), dtype=np.float32)
    CH = 2048
    for r0 in range(0, N, CH):
        r1 = min(r0 + CH, N)
        kk = keep[r0:r1]
        nkeep = int(kk.sum(axis=1).max())
        order_w = np.argsort(~kk, axis=1, kind="stable")[:, :nkeep]
        wpos_k = np.take_along_axis(wpos[r0:r1], order_w, axis=1)
        cs = np.sort(_expand_origins(wpos_k), axis=1)
        dd = dst[cs]
        dot32 = np.matmul(dd, src[r0:r1, :, None])[:, :, 0].astype(np.float32)
        score32 = (s2[r0:r1, None] + d2[cs]) - 2.0 * dot32
        mM = min(M, score32.shape[1] - 1)
        topM = np.argpartition(score32, mM, axis=1)[:, :mM]
        topM.sort(axis=1)
        csM = np.take_along_axis(cs, topM, axis=1)
        ddM = dst[csM]
        dot64 = np.matmul(ddM.astype(np.float64), src[r0:r1, :, None].astype(np.float64))[:, :, 0]
        distM = (s2_64[r0:r1, None] + d2_64[csM]) - 2.0 * dot64
        order = np.argsort(distM, axis=1, kind="stable")[:, :k]
        sel[r0:r1] = np.take_along_axis(csM, order, axis=1)
        dot32M = np.matmul(ddM, src[r0:r1, :, None])[:, :, 0].astype(np.float32)
        lik[r0:r1] = np.take_along_axis(dot32M, order, axis=1)

    lik = lik.reshape(-1)
    mu = np.float32(lik.mean())
    var = np.float32(lik.var())
    logits = (lik - mu) * np.float32(1.0 / np.sqrt(var + np.float32(EPS))) * gamma + beta
    w = (1.0 / (1.0 + np.exp(-logits))).astype(np.float32)
    w = (w / np.float32(w.mean())).astype(np.float32)[:, None]

    graph = np.stack(
        [
            np.repeat(np.arange(N, dtype=np.int32), k),
            sel.reshape(-1).astype(np.int32),
        ],
        axis=0,
    )
    return graph, w
